# revision 110
# baseline (speedup 1.0000x reference)
"""MLA forward kernel for Trainium2, 8 NeuronCores.

Sharding: data-parallel over batch (2) x tensor-parallel over heads (16 -> 4
groups of 4). Core c handles batch c//4, head group c%4. kv compression is
replicated per core. Each core emits a partial [S, D] output (its heads'
contribution through out_proj) in bf16; the host sums the 4 partials per batch
in fp32.

Design notes (single fused pipeline, per 512-row s-super):
  - x-side projections (kv latent, q_nope, q_rope) run in fp8e4 hi/lo
    "DoubleRow" matmuls: both operands split into fp8 hi + lo parts
    (host-prepped), 3 products hi*hi + lo*hi + hi*lo accumulate in PSUM.
    Weights are pre-scaled by 32 so the lo parts stay above the fp8
    subnormal floor; the scale cancels in rmsnorm (kv) or is folded into
    the softmax exp scale (q side).
  - rmsnorm rstd via act-Square(accum)/act-Sqrt + DVE reciprocal; the
    Square/Sqrt/Copy and Exp/Copy activation-table sets alternate only
    once per pipeline stage.
  - attention uses 128-wide q tiles (exact causal block-triangle).
    scores^T tiles [k=128, q=128]; P^T = exp(scale*(s+mask)) in bf16.
  - AV and the softmax row-sum are fused into ONE matmul per k-tile by
    augmenting V with a ones-column: out[q, 0:128] = attn numerator,
    out[q, 128] = sum(exp). Normalization is then a per-partition
    tensor_scalar multiply; no broadcast matmuls, no [1,N] row-sum
    matmuls.
  - attention output is produced q-major [q, dv], normalized, then
    PE-transposed to [dv, q] for the row-parallel out_proj.
  - PSUM budget (8 banks): 'big' [128,512] x3 (kv/qn/qr/kr/V/out_proj),
    'ss' [128,128] x3 (scores, Kn, all transposes), 'av' [128,132] x2.
  - rope rotations are split across DVE and the (otherwise idle)
    GPSIMD/Pool engine from an SBUF copy of the psum tile; PSUM->SBUF
    copies are balanced across Act and DVE. out_proj also runs as fp8
    hi/lo DoubleRow (head-pairs packed per matmul, weights pre-scaled by
    32, host divides the gathered partials).
"""

import sys
import numpy as np
import ml_dtypes

sys.path.insert(0, "/opt/trn_rl_repo")

import concourse.bass as bass  # noqa: E402
import concourse.tile as tile  # noqa: E402
from concourse import mybir, bacc  # noqa: E402
from concourse.bass_utils import run_bass_kernel_spmd  # noqa: E402
from concourse.masks import make_identity  # noqa: E402
from concourse.alu_op_type import AluOpType  # noqa: E402
from contextlib import ExitStack  # noqa: E402

B, S, D = 2, 2048, 2048
H, DN, DR, DV, R = 16, 128, 64, 128, 512
HL = 4  # heads per core
EPS = 1e-6
WS = 32.0  # fp8 weight pre-scale
SCALE = 1.0 / (float(np.sqrt(DN + DR)) * WS)
BF = mybir.dt.bfloat16
F32 = mybir.dt.float32
F8 = mybir.dt.float8e4
NT = S // 128   # 16 s-tiles
NJ = 4          # s-supers of 512
KP = 8          # D packs of 256
DRM = mybir.MatmulPerfMode.DoubleRow
AT = mybir.ActivationFunctionType

ROPE_ON_POOL = True

_CACHE = {}


def _build():
    nc = bacc.Bacc("TRN2", target_bir_lowering=False, debug=False)

    def din(name, shape, dt):
        return nc.dram_tensor(name, list(shape), dt, kind="ExternalInput").ap()

    xh_d = din("xh", [NJ, 4, 128, KP, 2, 128], F8)
    xl_d = din("xl", [NJ, 4, 128, KP, 2, 128], F8)
    wkvh_d = din("wkvh", [128, KP, 2, R], F8)
    wkvl_d = din("wkvl", [128, KP, 2, R], F8)
    wqnh_d = din("wqnh", [128, KP, 2, HL * DN], F8)
    wqnl_d = din("wqnl", [128, KP, 2, HL * DN], F8)
    wqrh_d = din("wqrh", [128, KP, 2, HL * DR], F8)
    wqrl_d = din("wqrl", [128, KP, 2, HL * DR], F8)
    wkn_d = din("wkn", [128, 4, HL * DN], BF)
    wkr_d = din("wkr", [128, 4, HL * DR], BF)
    wv_d = din("wv", [128, 4, HL * DV], BF)
    woh_d = din("woh", [2, 128, 2, D], F8)
    wol_d = din("wol", [2, 128, 2, D], F8)
    cos_d = din("cosb", [128, NT, 128], BF)
    sin_d = din("sinb", [128, NT, 128], BF)
    out_d = nc.dram_tensor("out", [S, D], BF, kind="ExternalOutput").ap()

    with tile.TileContext(nc) as tc, ExitStack() as outer:
        pp = outer.enter_context(tc.tile_pool(name="persist", bufs=1))
        ident = pp.tile([128, 128], BF, tag="ident", name="ident")
        bmask = pp.tile([128, 128], BF, tag="bmask", name="bmask")
        epst = pp.tile([128, 1], F32, tag="epst", name="epst")
        QnT = [pp.tile([128, S], BF, tag=f"QnT{h}", name=f"QnT{h}") for h in range(HL)]
        KnT = [pp.tile([128, S], BF, tag=f"KnT{h}", name=f"KnT{h}") for h in range(HL)]
        QrT = [pp.tile([128, S], BF, tag=f"QrT{r}", name=f"QrT{r}") for r in range(2)]
        KrT = [pp.tile([128, S], BF, tag=f"KrT{r}", name=f"KrT{r}") for r in range(2)]
        kvT = [pp.tile([128, S], BF, tag=f"kvT{r}", name=f"kvT{r}") for r in range(4)]
        outT_map = {}
        Vg = [pp.tile([128, HL, DV + 1], BF, tag=f"Vg{t}", name=f"Vg{t}")
              for t in range(NT)]
        woh_t = [pp.tile([128, 2, D], F8, tag=f"woh{p}", name=f"woh{p}")
                 for p in range(2)]
        wol_t = [pp.tile([128, 2, D], F8, tag=f"wol{p}", name=f"wol{p}")
                 for p in range(2)]
        wkn_t = pp.tile([128, 4, HL * DN], BF, tag="wkn", name="wkn")
        wkr_t = pp.tile([128, 4, HL * DR], BF, tag="wkr", name="wkr")
        wv_t = pp.tile([128, 4, HL * DV], BF, tag="wv", name="wv")
        wkvh_t = pp.tile([128, KP, 2, R], F8, tag="wkvh", name="wkvh")
        wkvl_t = pp.tile([128, KP, 2, R], F8, tag="wkvl", name="wkvl")
        wqnh_t = pp.tile([128, KP, 2, HL * DN], F8, tag="wqnh", name="wqnh")
        wqnl_t = pp.tile([128, KP, 2, HL * DN], F8, tag="wqnl", name="wqnl")
        wqrh_t = pp.tile([128, KP, 2, HL * DR], F8, tag="wqrh", name="wqrh")
        wqrl_t = pp.tile([128, KP, 2, HL * DR], F8, tag="wqrl", name="wqrl")
        ct = pp.tile([128, NT, 128], BF, tag="ct", name="ct")
        st = pp.tile([128, NT, 128], BF, tag="st", name="st")

        px = outer.enter_context(tc.tile_pool(name="xp", bufs=2))
        pw = outer.enter_context(tc.tile_pool(name="work", bufs=2))
        pbig = outer.enter_context(tc.tile_pool(name="pbig", bufs=3, space="PSUM"))
        pss = outer.enter_context(tc.tile_pool(name="pss", bufs=3, space="PSUM"))
        pav = outer.enter_context(tc.tile_pool(name="pav", bufs=2, space="PSUM"))

        # ---- constants
        nc.vector.memset(epst[:], float(WS * WS * EPS))
        make_identity(nc, ident[:])
        nc.gpsimd.memset(bmask[:], 1.0)
        # keep (1) where k <= q, i.e. (-part + col) >= 0; else 0
        nc.gpsimd.affine_select(
            out=bmask[:], in_=bmask[:], compare_op=AluOpType.is_ge,
            fill=0.0, base=0, pattern=[[1, 128]], channel_multiplier=-1)
        for t in range(NT):
            nc.gpsimd.memset(Vg[t][:, :, DV:DV + 1], 1.0)

        # ---- input DMAs, first-needed first
        xh_t = [None] * NJ
        xl_t = [None] * NJ

        def fetch_x(j):
            xh_t[j] = px.tile([128, 4, KP, 2, 128], F8, tag="xh", name=f"xh{j}")
            xl_t[j] = px.tile([128, 4, KP, 2, 128], F8, tag="xl", name=f"xl{j}")
            if j == 0:
                nc.sync.dma_start(wqrh_t[:], wqrh_d[:])
            # t4-major layout: per-s-tile transfers are fully dense;
            # hi/lo interleaved per tile to match the kv term order
            for t4 in range(4):
                nc.sync.dma_start(xh_t[j][:, t4], xh_d[j][t4])
                nc.sync.dma_start(xl_t[j][:, t4], xl_d[j][t4])

        fetch_x(0)
        nc.sync.dma_start(wqrl_t[:], wqrl_d[:])
        nc.sync.dma_start(wkvh_t[:], wkvh_d[:])
        nc.sync.dma_start(wkvl_t[:], wkvl_d[:])
        nc.sync.dma_start(ct[:], cos_d[:])
        nc.sync.dma_start(st[:], sin_d[:])
        nc.sync.dma_start(wqnh_t[:], wqnh_d[:])
        nc.sync.dma_start(wqnl_t[:], wqnl_d[:])
        nc.sync.dma_start(wkn_t[:], wkn_d[:])
        nc.sync.dma_start(wkr_t[:], wkr_d[:])
        nc.sync.dma_start(wv_t[:], wv_d[:])
        for p in range(2):
            nc.sync.dma_start(woh_t[p][:], woh_d[p])
            nc.sync.dma_start(wol_t[p][:], wol_d[p])

        # ---- deferred-emission queues (PE ops that follow Act/DVE results).
        # defer: transposes etc.; defer_o: out_proj chunks, drained with an
        # extra lag so the fp8 hi/lo splits of outT have time to complete.
        defer = []
        defer_o = []
        pending = []  # deferred attention-pair tails
        tcnt = {}

        def pump(n=1):
            for _ in range(min(n, len(pending))):
                pending.pop(0)()

        def drain(n=1):
            for _ in range(min(n, len(defer))):
                defer.pop(0)()

        def drain_o(n=1):
            for _ in range(min(n, len(defer_o))):
                i = defer_o[0][0]
                while tcnt.get(i, 0) < HL:
                    if pending:
                        pending.pop(0)()
                    elif defer:
                        defer.pop(0)()
                    else:
                        break
                defer_o.pop(0)[1]()

        def drain_all():
            pump(len(pending))
            drain(len(defer))
            drain_o(len(defer_o))

        def rope_chain(src, t, dstT):
            """src: [128,256] f32 psum (s-rows, 4 heads x 64 interleaved-pair
            rope dims). Writes rotated bf16 into dstT[r2][:, 128t:...] via
            deferred PE transposes."""
            rk = pw.tile([128, 256], F32, tag="rk", name="rk")
            nc.vector.tensor_copy(rk[:], src)
            rp = pw.tile([128, 256], BF, tag="rp", name="rp")
            e = rk[:, 0:256:2]
            o = rk[:, 1:256:2]
            de = rp[:, 0:256:2]
            do = rp[:, 1:256:2]
            cs = ct[:, t, :]
            sn = st[:, t, :]
            # split across DVE and Pool to halve the chain latency
            t1 = pw.tile([128, 128], F32, tag="t1", name="t1")
            t2 = pw.tile([128, 128], F32, tag="t2", name="t2")
            nc.vector.tensor_mul(t1[:], e, cs)
            nc.vector.tensor_mul(t2[:], o, sn)
            nc.vector.tensor_sub(de, t1[:], t2[:])
            t3 = pw.tile([128, 128], F32, tag="t3", name="t3")
            t4 = pw.tile([128, 128], F32, tag="t4", name="t4")
            nc.gpsimd.tensor_mul(t3[:], e, sn)
            nc.gpsimd.tensor_mul(t4[:], o, cs)
            nc.gpsimd.tensor_add(do, t3[:], t4[:])

            def emit(rp=rp, t=t, dstT=dstT):
                for r2 in range(2):
                    pt = pss.tile([128, 128], BF, tag="ss", name="ptr")
                    nc.tensor.transpose(pt[:], rp[:, 128 * r2:128 * (r2 + 1)],
                                        ident[:])
                    nc.scalar.copy(dstT[r2][:, 128 * t:128 * (t + 1)], pt[:])
            defer.append(emit)

        def _new_st(i, h):
            return {"h": h, "av": pav.tile([128, 132], F32, tag="av", name="av"),
                    "first": True, "avq": []}

        def _pop_av(st, n, last=False):
            for _ in range(min(n, len(st["avq"]))):
                K, ptsl = st["avq"].pop(0)
                fin = last and not st["avq"]
                nc.tensor.matmul(st["av"][:, 0:DV + 1], ptsl,
                                 Vg[K][:, st["h"], :],
                                 start=st["first"], stop=fin)
                st["first"] = False

        def _emit_group(i, st, k0, gsz):
            h = st["h"]
            ro = 64 * (h % 2)
            qsl = slice(128 * i, 128 * (i + 1))
            ps = pss.tile([128, 512], F32, tag="ss", name="ss")
            for u in range(gsz):
                K = k0 + u
                csl = slice(128 * u, 128 * (u + 1))
                ksl = slice(128 * K, 128 * (K + 1))
                nc.tensor.matmul(ps[:, csl], KnT[h][:, ksl], QnT[h][:, qsl],
                                 start=True, stop=False)
                nc.tensor.matmul(ps[:, csl], KrT[h // 2][ro:ro + 64, ksl],
                                 QrT[h // 2][ro:ro + 64, qsl],
                                 start=False, stop=True)
                if len(st["avq"]) > 3:  # ~2-group skew behind the exps
                    _pop_av(st, 1)
                if u == 1:
                    drain(1)
                elif u & 1:
                    drain_o(1)
            pt = pw.tile([128, 512], BF, tag="pT", bufs=4, name="pt")
            w = 128 * gsz
            nc.scalar.activation(pt[:, 0:w], ps[:, 0:w], AT.Exp, scale=SCALE)
            if k0 + gsz == i + 1:  # zero masked-out weights on diagonal unit
                dsl = slice(128 * (gsz - 1), 128 * gsz)
                nc.vector.tensor_mul(pt[:, dsl], pt[:, dsl], bmask[:])
            for u in range(gsz):
                st["avq"].append((k0 + u, pt[:, 128 * u:128 * (u + 1)]))

        def _tail(i, st):
            _pop_av(st, len(st["avq"]), last=True)
            h = st["h"]
            av = st["av"]
            rinv = pw.tile([128, 1], F32, tag="rinv", bufs=4, name="rinv")
            nc.vector.reciprocal(rinv[:], av[:, DV:DV + 1])
            oq = pw.tile([128, 128], BF, tag="oq", bufs=4, name="oq")
            nc.vector.tensor_scalar_mul(oq[:], av[:, 0:DV], rinv[:])

            def emit(i=i, h=h, oq=oq):
                pt = pss.tile([128, 128], BF, tag="ss", name="pto")
                nc.tensor.transpose(pt[:], oq[:], ident[:])
                if h == 0:
                    outT_map[i] = (
                        pw.tile([128, HL, 128], F8, tag="oth", bufs=2,
                                name="oth"),
                        pw.tile([128, HL, 128], F8, tag="otl", bufs=2,
                                name="otl"))
                oth, otl = outT_map[i]
                nc.vector.tensor_copy(oth[:, h, :], pt[:])
                nc.vector.tensor_sub(otl[:, h, :], pt[:], oth[:, h, :])
                tcnt[i] = tcnt.get(i, 0) + 1
            defer.append(emit)

        def attn_pair(i, h0, h1):
            sts = [_new_st(i, h0), _new_st(i, h1)]
            nu = i + 1
            first = True
            for k0 in range(0, nu, 4):
                gsz = min(4, nu - k0)
                for st in sts:
                    _emit_group(i, st, k0, gsz)
                if first:  # previous pair's tails overlap our first group
                    pump(2)
                first = False
            pending.append(lambda: _tail(i, sts[0]))
            pending.append(lambda: _tail(i, sts[1]))

        def op_chunk(i, dsl):
            ps = pbig.tile([128, 512], F32, tag="big", name="psf")
            dcs = slice(512 * dsl, 512 * (dsl + 1))
            isl = slice(128 * i, 128 * (i + 1))
            oth, otl = outT_map[i]
            n = 0
            for p in range(2):
                psl = slice(2 * p, 2 * p + 2)
                for (oa, wb) in ((oth, woh_t[p]), (otl, woh_t[p]),
                                 (oth, wol_t[p])):
                    nc.tensor.matmul(ps[:], oa[:, psl, :], wb[:, :, dcs],
                                     start=(n == 0), stop=(n == 5),
                                     perf_mode=DRM)
                    n += 1
            fo = pw.tile([128, 512], BF, tag="fo", bufs=3, name="fo")
            nc.vector.tensor_copy(fo[:], ps[:])
            nc.sync.dma_start(out_d[isl, dcs], fo[:])

        # ================= fused per-super pipeline =================
        for j in range(NJ):
            if j + 1 < NJ:
                fetch_x(j + 1)
            xh = xh_t[j]
            xl = xl_t[j]

            # ---- stage A: q_rope first (rope chains overlap the rest),
            # then latent compression, then q_nope
            for t4 in range(4):
                t = 4 * j + t4
                ssl = slice(128 * t4, 128 * (t4 + 1))
                ps = pbig.tile([128, 512], F32, tag="big", name="pqr")
                n = 0
                for (xa, wb) in ((xh, wqrh_t), (xl, wqrh_t), (xh, wqrl_t)):
                    for k in range(KP):  # hi*hi first: starts on partial DMA
                        nc.tensor.matmul(ps[:, 0:256], xa[:, t4, k, :, :],
                                         wb[:, k, :, :],
                                         start=(n == 0), stop=(n == 3 * KP - 1),
                                         perf_mode=DRM)
                        n += 1
                drain(2)
                rope_chain(ps[:, 0:256], t, QrT)

            def rms_chain(ps, t):
                sq = pw.tile([128, 512], BF, tag="sq", bufs=1, name="sq")
                var = pw.tile([128, 1], F32, tag="var", name="var")
                nc.scalar.activation(sq[:], ps[:], AT.Square, accum_out=var[:])
                std = pw.tile([128, 1], F32, tag="std", name="std")
                nc.scalar.activation(std[:], var[:], AT.Sqrt,
                                     bias=epst[:], scale=1.0 / R)
                rstd = pw.tile([128, 1], F32, tag="rstd", name="rstd")
                nc.vector.reciprocal(rstd[:], std[:])
                kvn = pw.tile([128, 512], BF, tag="kvn", name="kvn")
                nc.vector.tensor_scalar_mul(kvn[:], ps[:], rstd[:])

                def emit(kvn=kvn, t=t):
                    for r in range(4):
                        pt = pss.tile([128, 128], BF, tag="ss", name="ptk")
                        nc.tensor.transpose(pt[:], kvn[:, 128 * r:128 * (r + 1)],
                                            ident[:])
                        nc.scalar.copy(kvT[r][:, 128 * t:128 * (t + 1)], pt[:])
                defer.append(emit)

            kv_terms = ((xh, wkvh_t), (xl, wkvh_t), (xh, wkvl_t))
            for t4 in range(4):  # kv latent + rmsnorm, per-tile
                t = 4 * j + t4
                ps = pbig.tile([128, 512], F32, tag="big", name="pkv")
                n = 0
                for (xa, wb) in kv_terms:
                    for k in range(KP):
                        nc.tensor.matmul(ps[:], xa[:, t4, k, :, :],
                                         wb[:, k, :, :],
                                         start=(n == 0), stop=(n == 3 * KP - 1),
                                         perf_mode=DRM)
                        n += 1
                drain(4)
                drain_o(1)
                rms_chain(ps, t)

            for h in range(HL):  # q_nope, per s-tile (t4-major x layout)
                hsl = slice(128 * h, 128 * (h + 1))
                for t4 in range(4):
                    ps = pss.tile([128, 512], F32, tag="ss", name="pqn")
                    n = 0
                    for k in range(KP):
                        for (wa, xb) in ((wqnh_t, xh), (wqnl_t, xh),
                                         (wqnh_t, xl)):
                            nc.tensor.matmul(ps[:, 0:128], wa[:, k, :, hsl],
                                             xb[:, t4, k, :, :],
                                             start=(n == 0),
                                             stop=(n == 3 * KP - 1),
                                             perf_mode=DRM)
                            n += 1
                    if t4 == 1:
                        drain(2)
                        drain_o(1)
                    t = 4 * j + t4
                    nc.vector.tensor_copy(
                        QnT[h][:, 128 * t:128 * (t + 1)], ps[:, 0:128])

            # ---- stage B: latent up-projections for this super's k-tiles.
            # k_rope first so its Pool/DVE rope chains overlap the Kn/V
            # matmuls instead of stalling the first scores.
            drain_all()
            for t4 in range(4):
                t = 4 * j + t4
                tsl = slice(128 * t, 128 * (t + 1))
                ps = pbig.tile([128, 512], F32, tag="big", name="pkr")
                for r in range(4):
                    nc.tensor.matmul(ps[:, 0:256], kvT[r][:, tsl],
                                     wkr_t[:, r, :], start=(r == 0),
                                     stop=(r == 3))
                drain(1)
                rope_chain(ps[:, 0:256], t, KrT)
            for t4 in range(4):
                t = 4 * j + t4
                tsl = slice(128 * t, 128 * (t + 1))
                for h in range(HL):  # k_nope -> KnT
                    hsl = slice(128 * h, 128 * (h + 1))
                    ps = pss.tile([128, 128], F32, tag="ss", name="pkn")
                    for r in range(4):
                        nc.tensor.matmul(ps[:], wkn_t[:, r, hsl],
                                         kvT[r][:, tsl],
                                         start=(r == 0), stop=(r == 3))
                    drain(1)
                    nc.vector.tensor_copy(KnT[h][:, tsl], ps[:])
                psv = pbig.tile([128, 512], F32, tag="big", name="pv")
                for r in range(4):  # V
                    nc.tensor.matmul(psv[:], kvT[r][:, tsl], wv_t[:, r, :],
                                     start=(r == 0), stop=(r == 3))
                drain(1)
                nc.vector.tensor_copy(Vg[t][:, :, 0:DV], psv[:])

            # warm the Exp activation table while Act is idle, off the
            # critical path of the first score group's exp
            dmt = pw.tile([128, 1], F32, tag="dmt", bufs=1, name="dmt")
            nc.scalar.activation(dmt[:], epst[:], AT.Exp)

            # ---- stage C: attention + out_proj for q-tiles of this super
            drain_all()
            for t4 in range(4):
                i = 4 * j + t4
                attn_pair(i, 0, 1)
                attn_pair(i, 2, 3)
                for dsl in range(4):
                    defer_o.append((i, lambda i=i, dsl=dsl: op_chunk(i, dsl)))
        drain_all()

    nc.compile()
    return nc


def _f8_hilo(a):
    f8 = ml_dtypes.float8_e4m3
    hi = a.astype(f8)
    lo = (a - hi.astype(np.float32)).astype(f8)
    return hi, lo


def _prep_inputs(x, freqs, w_kv, g_kv, w_k, w_v, w_qn, w_qr, w_o):
    bf = ml_dtypes.bfloat16
    f32 = np.float32

    def pack_x(a):  # [D, S] -> [NJ, 4, 128, KP, 2, 128] (t4-major, dense)
        return np.ascontiguousarray(
            a.reshape(KP, 2, 128, NJ, 4, 128).transpose(3, 4, 2, 0, 1, 5))

    def pack_w(a):  # [D, C] -> [128, KP, 2, C]
        return np.ascontiguousarray(
            a.reshape(KP, 2, 128, a.shape[1]).transpose(2, 0, 1, 3))

    def pack_r(a):  # [R, C] -> [128, 4, C]
        return np.ascontiguousarray(
            a.reshape(4, 128, a.shape[1]).transpose(1, 0, 2))

    wk3 = (w_k.astype(f32) * g_kv.astype(f32)[:, None]).reshape(R, H, DN + DR)
    wv2 = (w_v.astype(f32) * g_kv.astype(f32)[:, None]).reshape(R, H, DV)

    # rope tables: packed e-view col c (of 128) has angle freqs[s, c % 32]
    ang = freqs.astype(f32)  # [S, 32]
    idx = np.tile(np.arange(32), 4)
    cos4 = np.cos(ang)[:, idx].reshape(NT, 128, 128).transpose(1, 0, 2)
    sin4 = np.sin(ang)[:, idx].reshape(NT, 128, 128).transpose(1, 0, 2)
    cosb = np.ascontiguousarray(cos4).astype(bf)
    sinb = np.ascontiguousarray(sin4).astype(bf)

    wkvh, wkvl = _f8_hilo(w_kv.astype(f32) * WS)
    in_maps = []
    for c in range(8):
        b, g = c // 4, c % 4
        hs = slice(4 * g, 4 * g + 4)
        xT = np.ascontiguousarray(x[b].astype(f32).T)  # [D, S]
        xh, xl = _f8_hilo(xT)
        wqn_c = np.ascontiguousarray(
            w_qn.reshape(D, H, DN)[:, hs].reshape(D, HL * DN)).astype(f32) * WS
        wqr_c = np.ascontiguousarray(
            w_qr.reshape(D, H, DR)[:, hs].reshape(D, HL * DR)).astype(f32) * WS
        wqnh, wqnl = _f8_hilo(wqn_c)
        wqrh, wqrl = _f8_hilo(wqr_c)
        m = {
            "xh": pack_x(xh), "xl": pack_x(xl),
            "wkvh": pack_w(wkvh), "wkvl": pack_w(wkvl),
            "wqnh": pack_w(wqnh), "wqnl": pack_w(wqnl),
            "wqrh": pack_w(wqrh), "wqrl": pack_w(wqrl),
            "wkn": pack_r(np.ascontiguousarray(
                wk3[:, hs, :DN].reshape(R, HL * DN)).astype(bf)),
            "wkr": pack_r(np.ascontiguousarray(
                wk3[:, hs, DN:].reshape(R, HL * DR)).astype(bf)),
            "wv": pack_r(np.ascontiguousarray(
                wv2[:, hs].reshape(R, HL * DV)).astype(bf)),
            "cosb": cosb, "sinb": sinb,
        }
        wo4 = w_o.reshape(H, DV, D)[hs].astype(f32) * WS
        f8t = ml_dtypes.float8_e4m3
        m["woh"] = np.empty((2, 128, 2, D), f8t)
        m["wol"] = np.empty((2, 128, 2, D), f8t)
        for p in range(2):
            wop = np.ascontiguousarray(
                wo4[2 * p:2 * p + 2].transpose(1, 0, 2))
            m["woh"][p], m["wol"][p] = _f8_hilo(wop)
        in_maps.append(m)
    return in_maps


def kernel(x, freqs, w_kv, g_kv, w_k, w_v, w_qn, w_qr, w_o):
    if "nc" not in _CACHE:
        _CACHE["nc"] = _build()
    nc = _CACHE["nc"]
    in_maps = _prep_inputs(np.asarray(x), np.asarray(freqs), np.asarray(w_kv),
                           np.asarray(g_kv), np.asarray(w_k), np.asarray(w_v),
                           np.asarray(w_qn), np.asarray(w_qr), np.asarray(w_o))
    res = run_bass_kernel_spmd(nc, in_maps, list(range(8)), trace=False)
    out = np.zeros((B, S, D), np.float32)
    for c in range(8):
        out[c // 4] += res.results[c]["out"].astype(np.float32)
    out /= WS  # out_proj weights were pre-scaled for fp8
    return out
